# revision 5
# baseline (speedup 1.0000x reference)
"""Trainium2 Bass kernel for CompiledNCA (stem conv -> 16 NCA conv steps -> pool -> fc).

Strategy: pure data parallel over batch (128 images -> 8 cores x 16 images).
Per core, images are processed in 2 groups of 8. SBUF layout puts
(image, channel) on the 128 partitions and the zero-padded 130x130 image
(flattened) on the free dim, so every 3x3 tap is a free-dim offset read.
Each conv step is 9 PSUM-accumulated matmuls with block-diagonal (8 x [16x16])
fp16 weights, using the full 128x128 PE array for 8 images at once.
ReLU + fp32->fp16 cast on ScalarE writes only interior pixels, so the zero
halo installed by an initial memset survives all steps. The final step's
activations also emit accum_out partial sums (free spatial pooling); the fc
layer is one tiny block-diagonal fp32 matmul + bias add.
"""

import numpy as np

B, HC, OC, T = 128, 16, 10, 16
H = W = 128
PW = H + 2                     # padded width/height: 130
S = PW * PW                    # 16900 flat padded pixels
NINT = H * PW                  # 16640 columns covering interior rows 1..128
GUARD = 4                      # slack so shifted matmul reads stay in-bounds
HBUF = GUARD + S + 12          # h buffer free size (reads reach GUARD+S)
NCORES = 8
IMGS = 8                       # images per group (block-diag batch)
GROUPS = 2                     # groups per core
ROWS_PER_TILE = 3              # 3 image rows -> N=390 <= 512 (one PSUM bank)
Y0S = list(range(1, H + 1, ROWS_PER_TILE))   # 43 row-tiles over rows 1..128
NTILES = len(Y0S)

_TAP_OFF = [(dy - 1) * PW + (dx - 1) for dy in range(3) for dx in range(3)]

_cache = {}


def _build(repeat=1):
    """Build + compile the Bass graph once; cached across kernel() calls.

    repeat > 1 duplicates the whole compute (same inputs/outputs) and is
    used only by the timing harness to measure marginal per-iteration
    device time within a single NEFF launch."""
    if repeat in _cache:
        return _cache[repeat]

    import concourse.bacc as bacc
    import concourse.mybir as mybir
    import concourse.tile as tile

    f16, f32 = mybir.dt.float16, mybir.dt.float32
    Relu = mybir.ActivationFunctionType.Relu

    nc = bacc.Bacc("TRN2", target_bir_lowering=False, debug=False,
                   enable_asserts=False, num_devices=NCORES)

    d_w = nc.dram_tensor("w_steps", [128, T * 9 * 128], f16, kind="ExternalInput")
    d_stemw = nc.dram_tensor("stem_w", [IMGS * 9, 128], f16, kind="ExternalInput")
    d_stemb = nc.dram_tensor("stem_b", [128, 1], f32, kind="ExternalInput")
    d_fcw = nc.dram_tensor("fc_w", [128, IMGS * OC], f32, kind="ExternalInput")
    d_fcb = nc.dram_tensor("fc_b", [IMGS * OC, 1], f32, kind="ExternalInput")
    d_xe = [nc.dram_tensor(f"x_exp{g}", [IMGS * 9, NINT], f16, kind="ExternalInput")
            for g in range(GROUPS)]
    d_out = nc.dram_tensor("out", [GROUPS * IMGS, OC], f32, kind="ExternalOutput")

    with tile.TileContext(nc) as tc:
        with tc.tile_pool(name="const", bufs=1) as cp, \
             tc.tile_pool(name="hbuf", bufs=1) as hp, \
             tc.tile_pool(name="small", bufs=2) as sp, \
             tc.tile_pool(name="psum", bufs=6, space="PSUM") as pp, \
             tc.tile_pool(name="psfc", bufs=2, space="PSUM") as pf:

            w_sb = cp.tile([128, T * 9 * 128], f16, tag="w")
            nc.sync.dma_start(w_sb[:], d_w[:])
            stemw_sb = cp.tile([IMGS * 9, 128], f16, tag="sw")
            nc.sync.dma_start(stemw_sb[:], d_stemw[:])
            stemb_sb = cp.tile([128, 1], f32, tag="sb")
            nc.sync.dma_start(stemb_sb[:], d_stemb[:])
            fcw_sb = cp.tile([128, IMGS * OC], f32, tag="fw")
            nc.sync.dma_start(fcw_sb[:], d_fcw[:])
            fcb_sb = cp.tile([IMGS * OC, 1], f32, tag="fb")
            nc.sync.dma_start(fcb_sb[:], d_fcb[:])
            xe_sb = []
            for g in range(GROUPS):
                t_ = cp.tile([IMGS * 9, NINT], f16, tag=f"xe{g}",
                             name=f"xe{g}")
                nc.sync.dma_start(t_[:], d_xe[g][:])
                xe_sb.append(t_)

            hbufs = [hp.tile([128, HBUF], f16, tag=f"h{i}", name=f"h{i}")
                     for i in range(2)]
            nc.vector.memset(hbufs[0][:], 0.0)
            nc.gpsimd.memset(hbufs[1][:], 0.0)

            for _rep in range(repeat):
              for g in range(GROUPS):
                acc = sp.tile([128, NTILES], f32, tag="acc")
                # t = -1 is the stem (im2col-expanded x, one matmul pass);
                # t in [0, T) are the NCA steps (9 taps each).
                for t in range(-1, T):
                    src = hbufs[t % 2]
                    dst = hbufs[(t + 1) % 2]
                    for r, y0 in enumerate(Y0S):
                        rows = min(ROWS_PER_TILE, H + 1 - y0)
                        N = rows * PW
                        ps = pp.tile([128, ROWS_PER_TILE * PW], f32, tag="ps")
                        if t < 0:
                            nc.tensor.matmul(
                                ps[:, :N], stemw_sb[:],
                                xe_sb[g][:, (y0 - 1) * PW:(y0 - 1) * PW + N],
                                start=True, stop=True)
                        else:
                            for tap in range(9):
                                col = GUARD + y0 * PW + _TAP_OFF[tap]
                                wv = w_sb[:, (t * 9 + tap) * 128:(t * 9 + tap + 1) * 128]
                                nc.tensor.matmul(
                                    ps[:, :N], wv, src[:, col:col + N],
                                    start=(tap == 0), stop=(tap == 8))
                        # interior-only relu copy: skips the 2 halo columns
                        # per image row so halos stay zero
                        src_ap = ps[:, :N].rearrange(
                            "p (r w) -> p r w", w=PW)[:, :, 1:1 + W]
                        d0 = GUARD + y0 * PW + 1
                        dst_ap = dst[:, d0:d0 + N].rearrange(
                            "p (r w) -> p r w", w=PW)[:, :, 0:W]
                        kw = {}
                        if t == T - 1:
                            kw["accum_out"] = acc[:, r:r + 1]
                        if t < 0:
                            nc.scalar.activation(dst_ap, src_ap, Relu,
                                                 bias=stemb_sb[:], **kw)
                        else:
                            nc.scalar.activation(dst_ap, src_ap, Relu, **kw)

                pooled = sp.tile([128, 1], f32, tag="pooled")
                nc.vector.tensor_reduce(pooled[:], acc[:],
                                        axis=mybir.AxisListType.X,
                                        op=mybir.AluOpType.add)
                psfc = pf.tile([IMGS * OC, 1], f32, tag="fc")
                nc.tensor.matmul(psfc[:], fcw_sb[:], pooled[:],
                                 start=True, stop=True)
                out_sb = sp.tile([IMGS * OC, 1], f32, tag="osb")
                nc.vector.tensor_add(out_sb[:], psfc[:], fcb_sb[:])
                nc.sync.dma_start(d_out[g * IMGS:(g + 1) * IMGS, :], out_sb[:])

    nc.compile()
    _cache[repeat] = nc
    return nc


def _prep_shared(stem_weight, stem_bias, weight_schedule, fc_weight, fc_bias):
    """Host-side packing of the (replicated) weight tensors into SBUF layouts."""
    # per-step per-tap block-diagonal lhsT: [128,(img,ci) , 128,(img,co)]
    w = weight_schedule.astype(np.float32)          # [T, co, ci, 3, 3]
    lhs = np.zeros((T, 9, 128, 128), np.float16)
    for tap in range(9):
        dy, dx = tap // 3, tap % 3
        blk = np.transpose(w[:, :, :, dy, dx], (0, 2, 1))  # [T, ci, co]
        for i in range(IMGS):
            lhs[:, tap, i * HC:(i + 1) * HC, i * HC:(i + 1) * HC] = blk
    w_steps = np.ascontiguousarray(
        np.transpose(lhs, (2, 0, 1, 3)).reshape(128, T * 9 * 128))

    sw = stem_weight.astype(np.float32)             # [HC, 1, 3, 3]
    stem_lhs = np.zeros((IMGS * 9, 128), np.float16)
    for tap in range(9):
        dy, dx = tap // 3, tap % 3
        for i in range(IMGS):
            stem_lhs[i * 9 + tap, i * HC:(i + 1) * HC] = sw[:, 0, dy, dx]

    stem_b = np.tile(stem_bias.astype(np.float32), IMGS)[:, None].copy()

    fcw = np.zeros((128, IMGS * OC), np.float32)
    for i in range(IMGS):
        fcw[i * HC:(i + 1) * HC, i * OC:(i + 1) * OC] = \
            fc_weight.astype(np.float32).T / float(H * W)
    fc_b = np.tile(fc_bias.astype(np.float32), IMGS)[:, None].copy()

    return {"w_steps": w_steps, "stem_w": stem_lhs, "stem_b": stem_b,
            "fc_w": fcw, "fc_b": fc_b}


def _prep_xexp(x_imgs):
    """im2col-expand 8 images for the stem: [(img,tap), NINT] fp16.

    x_exp[(i,tap), j] = xpad[i, j + PW + off_tap] (flat padded coords,
    out-of-range reads are zero)."""
    xpad = np.zeros((IMGS, PW, PW), np.float32)
    xpad[:, 1:1 + H, 1:1 + W] = x_imgs[:, 0]
    flat = np.pad(xpad.reshape(IMGS, S), ((0, 0), (132, 132)))
    out = np.empty((IMGS * 9, NINT), np.float16)
    for i in range(IMGS):
        for tap in range(9):
            base = 132 + PW + _TAP_OFF[tap]
            out[i * 9 + tap] = flat[i, base:base + NINT]
    return out


def kernel(x, stem_weight, stem_bias, weight_schedule, fc_weight, fc_bias):
    from concourse.bass_utils import run_bass_kernel_spmd

    nc = _build()
    shared = _prep_shared(stem_weight, stem_bias, weight_schedule,
                          fc_weight, fc_bias)
    in_maps = []
    for c in range(NCORES):
        m = dict(shared)
        for g in range(GROUPS):
            lo = c * GROUPS * IMGS + g * IMGS
            m[f"x_exp{g}"] = _prep_xexp(np.asarray(x[lo:lo + IMGS],
                                                   dtype=np.float32))
        in_maps.append(m)

    res = run_bass_kernel_spmd(nc, in_maps, core_ids=list(range(NCORES)),
                               trace=False)
    out = np.concatenate([res.results[c]["out"] for c in range(NCORES)], axis=0)
    return out.astype(np.float32)


# revision 8
# speedup vs baseline: 4.2861x; 4.2861x over previous
"""Trainium2 Bass kernel for CompiledNCA (stem conv -> 16 NCA conv steps -> pool -> fc).

Strategy: pure data parallel over batch (128 images -> 8 cores x 16 images).
Per core, images are processed in 2 groups of 8. SBUF layout puts
(image, channel) on the 128 partitions and the zero-padded 130x130 image
(flattened) on the free dim, so every 3x3 tap is a free-dim offset read.
Each conv step is 9 PSUM-accumulated matmuls with block-diagonal (8 x [16x16])
fp16 weights, using the full 128x128 PE array for 8 images at once.
ReLU + fp32->fp16 cast on ScalarE writes only interior pixels, so the zero
halo installed by an initial memset survives all steps. The final step's
activations also emit accum_out partial sums (free spatial pooling); the fc
layer is one tiny block-diagonal fp32 matmul + bias add.
"""

import numpy as np

B, HC, OC, T = 128, 16, 10, 16
H = W = 128
PW = H + 2                     # padded width/height: 130
S = PW * PW                    # 16900 flat padded pixels
NINT = H * PW                  # 16640 columns covering interior rows 1..128
XEW = NINT + 8                 # x_exp width (pad so 520-wide views stay in-bounds)
GUARD = 4                      # slack so shifted matmul reads stay in-bounds
HBUF = GUARD + S + 12          # h buffer free size (reads reach GUARD+S)
NCORES = 8
IMGS = 8                       # images per group (block-diag batch)
GROUPS = 2                     # groups per core
ROWS_PER_TILE = 4              # 4 rows x 128 interior cols -> N=512 (one bank)
Y0S = list(range(1, H + 1, ROWS_PER_TILE))   # 32 row-tiles over rows 1..128
NTILES = len(Y0S)
NT = ROWS_PER_TILE * W         # 512 matmul free size (interior only)

_TAP_OFF = [(dy - 1) * PW + (dx - 1) for dy in range(3) for dx in range(3)]

_cache = {}


def _build(repeat=1):
    """Build + compile the Bass graph once; cached across kernel() calls.

    repeat > 1 duplicates the whole compute (same inputs/outputs) and is
    used only by the timing harness to measure marginal per-iteration
    device time within a single NEFF launch."""
    if repeat in _cache:
        return _cache[repeat]

    import concourse.bacc as bacc
    import concourse.mybir as mybir
    import concourse.tile as tile

    f16, f32 = mybir.dt.float16, mybir.dt.float32
    Relu = mybir.ActivationFunctionType.Relu

    nc = bacc.Bacc("TRN2", target_bir_lowering=False, debug=False,
                   enable_asserts=False, num_devices=NCORES)

    d_w = nc.dram_tensor("w_steps", [128, T * 9 * 128], f16, kind="ExternalInput")
    d_stemw = nc.dram_tensor("stem_w", [IMGS * 9, 128], f16, kind="ExternalInput")
    d_stemb = nc.dram_tensor("stem_b", [128, 1], f32, kind="ExternalInput")
    d_fcw = nc.dram_tensor("fc_w", [128, IMGS * OC], f32, kind="ExternalInput")
    d_fcb = nc.dram_tensor("fc_b", [IMGS * OC, 1], f32, kind="ExternalInput")
    d_xe = [nc.dram_tensor(f"x_exp{g}", [IMGS * 9, XEW], f16, kind="ExternalInput")
            for g in range(GROUPS)]
    d_out = nc.dram_tensor("out", [GROUPS * IMGS, OC], f32, kind="ExternalOutput")

    with tile.TileContext(nc) as tc:
        with tc.tile_pool(name="const", bufs=1) as cp, \
             tc.tile_pool(name="hbuf", bufs=1) as hp, \
             tc.tile_pool(name="small", bufs=2) as sp, \
             tc.tile_pool(name="psum", bufs=6, space="PSUM") as pp, \
             tc.tile_pool(name="psfc", bufs=2, space="PSUM") as pf:

            w_sb = cp.tile([128, T * 9 * 128], f16, tag="w")
            nc.sync.dma_start(w_sb[:], d_w[:])
            stemw_sb = cp.tile([IMGS * 9, 128], f16, tag="sw")
            nc.sync.dma_start(stemw_sb[:], d_stemw[:])
            stemb_sb = cp.tile([128, 1], f32, tag="sb")
            nc.sync.dma_start(stemb_sb[:], d_stemb[:])
            fcw_sb = cp.tile([128, IMGS * OC], f32, tag="fw")
            nc.sync.dma_start(fcw_sb[:], d_fcw[:])
            fcb_sb = cp.tile([IMGS * OC, 1], f32, tag="fb")
            nc.sync.dma_start(fcb_sb[:], d_fcb[:])
            xe_sb = []
            for g in range(GROUPS):
                t_ = cp.tile([IMGS * 9, XEW], f16, tag=f"xe{g}",
                             name=f"xe{g}")
                nc.sync.dma_start(t_[:], d_xe[g][:])
                xe_sb.append(t_)

            hbufs = [hp.tile([128, HBUF], f16, tag=f"h{i}", name=f"h{i}")
                     for i in range(2)]
            nc.vector.memset(hbufs[0][:], 0.0)
            nc.gpsimd.memset(hbufs[1][:], 0.0)

            for _rep in range(repeat):
              for g in range(GROUPS):
                acc = sp.tile([128, NTILES], f32, tag="acc")
                # t = -1 is the stem (im2col-expanded x, one matmul pass);
                # t in [0, T) are the NCA steps (9 taps each).
                for t in range(-1, T):
                    src = hbufs[t % 2]
                    dst = hbufs[(t + 1) % 2]
                    for r, y0 in enumerate(Y0S):
                        RW = ROWS_PER_TILE * PW  # 520-col span for 3D views
                        ps = pp.tile([128, NT], f32, tag="ps")
                        if t < 0:
                            j0 = (y0 - 1) * PW + 1
                            rhs = xe_sb[g][:, j0:j0 + RW].rearrange(
                                "p (r w) -> p r w", w=PW)[:, :, 0:W]
                            nc.tensor.matmul(ps[:], stemw_sb[:], rhs,
                                             start=True, stop=True)
                        else:
                            for tap in range(9):
                                a = GUARD + y0 * PW + 1 + _TAP_OFF[tap]
                                rhs = src[:, a:a + RW].rearrange(
                                    "p (r w) -> p r w", w=PW)[:, :, 0:W]
                                wv = w_sb[:, (t * 9 + tap) * 128:(t * 9 + tap + 1) * 128]
                                nc.tensor.matmul(
                                    ps[:], wv, rhs,
                                    start=(tap == 0), stop=(tap == 8))
                        # interior-only relu copy: halos are never written so
                        # they stay zero from the initial memset
                        src_ap = ps[:].rearrange("p (r w) -> p r w", w=W)
                        d0 = GUARD + y0 * PW + 1
                        dst_ap = dst[:, d0:d0 + RW].rearrange(
                            "p (r w) -> p r w", w=PW)[:, :, 0:W]
                        kw = {}
                        if t == T - 1:
                            kw["accum_out"] = acc[:, r:r + 1]
                        if t < 0:
                            nc.scalar.activation(dst_ap, src_ap, Relu,
                                                 bias=stemb_sb[:], **kw)
                        else:
                            nc.scalar.activation(dst_ap, src_ap, Relu, **kw)

                pooled = sp.tile([128, 1], f32, tag="pooled")
                nc.vector.tensor_reduce(pooled[:], acc[:],
                                        axis=mybir.AxisListType.X,
                                        op=mybir.AluOpType.add)
                psfc = pf.tile([IMGS * OC, 1], f32, tag="fc")
                nc.tensor.matmul(psfc[:], fcw_sb[:], pooled[:],
                                 start=True, stop=True)
                out_sb = sp.tile([IMGS * OC, 1], f32, tag="osb")
                nc.vector.tensor_add(out_sb[:], psfc[:], fcb_sb[:])
                nc.sync.dma_start(d_out[g * IMGS:(g + 1) * IMGS, :], out_sb[:])

    nc.compile()
    _cache[repeat] = nc
    return nc


def _prep_shared(stem_weight, stem_bias, weight_schedule, fc_weight, fc_bias):
    """Host-side packing of the (replicated) weight tensors into SBUF layouts."""
    # per-step per-tap block-diagonal lhsT: [128,(img,ci) , 128,(img,co)]
    w = weight_schedule.astype(np.float32)          # [T, co, ci, 3, 3]
    lhs = np.zeros((T, 9, 128, 128), np.float16)
    for tap in range(9):
        dy, dx = tap // 3, tap % 3
        blk = np.transpose(w[:, :, :, dy, dx], (0, 2, 1))  # [T, ci, co]
        for i in range(IMGS):
            lhs[:, tap, i * HC:(i + 1) * HC, i * HC:(i + 1) * HC] = blk
    w_steps = np.ascontiguousarray(
        np.transpose(lhs, (2, 0, 1, 3)).reshape(128, T * 9 * 128))

    sw = stem_weight.astype(np.float32)             # [HC, 1, 3, 3]
    stem_lhs = np.zeros((IMGS * 9, 128), np.float16)
    for tap in range(9):
        dy, dx = tap // 3, tap % 3
        for i in range(IMGS):
            stem_lhs[i * 9 + tap, i * HC:(i + 1) * HC] = sw[:, 0, dy, dx]

    stem_b = np.tile(stem_bias.astype(np.float32), IMGS)[:, None].copy()

    fcw = np.zeros((128, IMGS * OC), np.float32)
    for i in range(IMGS):
        fcw[i * HC:(i + 1) * HC, i * OC:(i + 1) * OC] = \
            fc_weight.astype(np.float32).T / float(H * W)
    fc_b = np.tile(fc_bias.astype(np.float32), IMGS)[:, None].copy()

    return {"w_steps": w_steps, "stem_w": stem_lhs, "stem_b": stem_b,
            "fc_w": fcw, "fc_b": fc_b}


def _prep_xexp(x_imgs):
    """im2col-expand 8 images for the stem: [(img,tap), NINT] fp16.

    x_exp[(i,tap), j] = xpad[i, j + PW + off_tap] (flat padded coords,
    out-of-range reads are zero)."""
    xpad = np.zeros((IMGS, PW, PW), np.float32)
    xpad[:, 1:1 + H, 1:1 + W] = x_imgs[:, 0]
    flat = np.pad(xpad.reshape(IMGS, S), ((0, 0), (132, 132)))
    out = np.zeros((IMGS * 9, XEW), np.float16)
    for i in range(IMGS):
        for tap in range(9):
            base = 132 + PW + _TAP_OFF[tap]
            out[i * 9 + tap, :NINT] = flat[i, base:base + NINT]
    return out


def kernel(x, stem_weight, stem_bias, weight_schedule, fc_weight, fc_bias):
    from concourse.bass_utils import run_bass_kernel_spmd

    nc = _build()
    shared = _prep_shared(stem_weight, stem_bias, weight_schedule,
                          fc_weight, fc_bias)
    in_maps = []
    for c in range(NCORES):
        m = dict(shared)
        for g in range(GROUPS):
            lo = c * GROUPS * IMGS + g * IMGS
            m[f"x_exp{g}"] = _prep_xexp(np.asarray(x[lo:lo + IMGS],
                                                   dtype=np.float32))
        in_maps.append(m)

    res = run_bass_kernel_spmd(nc, in_maps, core_ids=list(range(NCORES)),
                               trace=False)
    out = np.concatenate([res.results[c]["out"] for c in range(NCORES)], axis=0)
    return out.astype(np.float32)


# revision 19
# speedup vs baseline: 4.4751x; 1.0441x over previous
"""Trainium2 Bass kernel for CompiledNCA (stem conv -> 16 NCA conv steps -> pool -> fc).

Strategy: pure data parallel over batch (128 images -> 8 cores x 16 images).
Per core, images are processed in 2 groups of 8. SBUF layout puts
(image, channel) on the 128 partitions and the zero-padded 130x130 image
(flattened) on the free dim, so every 3x3 tap is a free-dim offset read.
Each conv step is 9 PSUM-accumulated matmuls with block-diagonal (8 x [16x16])
fp16 weights, using the full 128x128 PE array for 8 images at once.
ReLU + fp32->fp16 cast on ScalarE writes only interior pixels, so the zero
halo installed by an initial memset survives all steps. The final step's
activations also emit accum_out partial sums (free spatial pooling); the fc
layer is one tiny block-diagonal fp32 matmul + bias add.
"""

import numpy as np

B, HC, OC, T = 128, 16, 10, 16
H = W = 128
PW = H + 2                     # padded width/height: 130
S = PW * PW                    # 16900 flat padded pixels
NINT = H * PW                  # 16640 columns covering interior rows 1..128
XEW = NINT + 8                 # x_exp width (pad so 520-wide views stay in-bounds)
GUARD = 4                      # slack so shifted matmul reads stay in-bounds
HBUF = GUARD + S + 12          # h buffer free size (reads reach GUARD+S)
NCORES = 8
IMGS = 8                       # images per group (block-diag batch)
GROUPS = 2                     # groups per core
ROWS_PER_TILE = 4              # 4 rows x 128 interior cols -> N=512 (one bank)
Y0S = list(range(1, H + 1, ROWS_PER_TILE))   # 32 row-tiles over rows 1..128
NTILES = len(Y0S)
NT = ROWS_PER_TILE * W         # 512 matmul free size (interior only)
XCH = 4                        # x_exp DMA chunks per group (per-chunk deps)
XCW = NINT // XCH              # 4160 cols per chunk (+8 overlap columns)

_TAP_OFF = [(dy - 1) * PW + (dx - 1) for dy in range(3) for dx in range(3)]

_cache = {}


def _build(repeat=1):
    """Build + compile the Bass graph once; cached across kernel() calls.

    repeat > 1 duplicates the whole compute (same inputs/outputs) and is
    used only by the timing harness to measure marginal per-iteration
    device time within a single NEFF launch."""
    if repeat in _cache:
        return _cache[repeat]

    import concourse.bacc as bacc
    import concourse.mybir as mybir
    import concourse.tile as tile

    f16, f32 = mybir.dt.float16, mybir.dt.float32
    Relu = mybir.ActivationFunctionType.Relu

    nc = bacc.Bacc("TRN2", target_bir_lowering=False, debug=False,
                   enable_asserts=False, num_devices=NCORES)

    d_w = nc.dram_tensor("w_steps", [128, T * 9 * 128], f16, kind="ExternalInput")
    d_stemw = nc.dram_tensor("stem_w", [IMGS * 9, 128], f16, kind="ExternalInput")
    d_stemb = nc.dram_tensor("stem_b", [128, 1], f32, kind="ExternalInput")
    d_fcw = nc.dram_tensor("fc_w", [128, IMGS * OC], f32, kind="ExternalInput")
    d_fcb = nc.dram_tensor("fc_b", [IMGS * OC, 1], f32, kind="ExternalInput")
    d_xe = [nc.dram_tensor(f"x_exp{g}", [IMGS * 9, XCH * (XCW + 8)], f16,
                           kind="ExternalInput")
            for g in range(GROUPS)]
    d_out = nc.dram_tensor("out", [GROUPS * IMGS, OC], f32, kind="ExternalOutput")

    with tile.TileContext(nc) as tc:
        with tc.tile_pool(name="const", bufs=1) as cp, \
             tc.tile_pool(name="hbuf", bufs=1) as hp, \
             tc.tile_pool(name="small", bufs=2) as sp, \
             tc.tile_pool(name="psum", bufs=8, space="PSUM") as pp:

            # DMA order matters for the startup stall: the stem needs only
            # stem_w/stem_b + the first x_exp chunk, so those go first and
            # get their own tiles (per-tile deps); the 4.7MB step-weight DMA
            # is split per step and queued behind.
            stemw_sb = cp.tile([IMGS * 9, 128], f16, tag="sw")
            nc.sync.dma_start(stemw_sb[:], d_stemw[:])
            stemb_sb = cp.tile([128, 1], f32, tag="sb")
            nc.sync.dma_start(stemb_sb[:], d_stemb[:])

            def load_xe_chunk(g, c):
                t_ = cp.tile([IMGS * 9, XCW + 8], f16, tag=f"xe{g}_{c}",
                             name=f"xe{g}_{c}")
                nc.sync.dma_start(
                    t_[:], d_xe[g][:, c * (XCW + 8):(c + 1) * (XCW + 8)])
                return t_

            def load_w(t):
                t_ = cp.tile([128, 9 * 128], f16, tag=f"w{t}", name=f"w{t}")
                nc.sync.dma_start(t_[:], d_w[:, t * 9 * 128:(t + 1) * 9 * 128])
                return t_

            # issue order tracks first-use time: stem consumes xe0 chunks
            # from ~2us, step 0 needs w0 at ~8us, step t at ~8+t*61us,
            # group 1's xe at ~1ms.
            xe_sb = [[load_xe_chunk(0, 0), load_xe_chunk(0, 1)], []]
            w_tiles = [load_w(0)]
            for c in range(2, XCH):
                xe_sb[0].append(load_xe_chunk(0, c))
            for t in range(1, T):
                w_tiles.append(load_w(t))
            for c in range(XCH):
                xe_sb[1].append(load_xe_chunk(1, c))
            fcw_sb = cp.tile([128, IMGS * OC], f32, tag="fw")
            nc.sync.dma_start(fcw_sb[:], d_fcw[:])
            fcb_sb = cp.tile([IMGS * OC, 1], f32, tag="fb")
            nc.sync.dma_start(fcb_sb[:], d_fcb[:])

            hbufs = [hp.tile([128, HBUF], f16, tag=f"h{i}", name=f"h{i}")
                     for i in range(2)]
            # zero only the cells the relu copies never write: guards, the
            # top/bottom halo rows, and the 2-col halo seams between rows.
            # The interior is overwritten every step, so a full-buffer memset
            # (~17us on DVE) would only stall the stem.
            for hb in hbufs:
                nc.vector.memset(hb[:, 0:GUARD + PW + 1], 0.0)
                s0 = GUARD + PW + W + 1          # right halo of row 1
                seams = hb[:, s0:s0 + (H - 1) * PW].rearrange(
                    "p (y two) -> p y two", two=PW)[:, :, 0:2]
                nc.vector.memset(seams, 0.0)
                nc.vector.memset(hb[:, GUARD + H * PW + PW - 1:HBUF], 0.0)

            for _rep in range(repeat):
              for g in range(GROUPS):
                acc = sp.tile([128, NTILES], f32, tag="acc")
                # t = -1 is the stem (im2col-expanded x, one matmul pass);
                # t in [0, T) are the NCA steps (9 taps each).
                for t in range(-1, T):
                    src = hbufs[t % 2]
                    dst = hbufs[(t + 1) % 2]
                    for r, y0 in enumerate(Y0S):
                        RW = ROWS_PER_TILE * PW  # 520-col span for 3D views
                        ps = pp.tile([128, NT], f32, tag="ps")
                        if t < 0:
                            c = r // (NTILES // XCH)
                            j0 = (y0 - 1) * PW + 1 - c * XCW
                            rhs = xe_sb[g][c][:, j0:j0 + RW].rearrange(
                                "p (r w) -> p r w", w=PW)[:, :, 0:W]
                            nc.tensor.matmul(ps[:], stemw_sb[:], rhs,
                                             start=True, stop=True)
                        else:
                            for tap in range(9):
                                a = GUARD + y0 * PW + 1 + _TAP_OFF[tap]
                                rhs = src[:, a:a + RW].rearrange(
                                    "p (r w) -> p r w", w=PW)[:, :, 0:W]
                                wv = w_tiles[t][:, tap * 128:(tap + 1) * 128]
                                nc.tensor.matmul(
                                    ps[:], wv, rhs,
                                    start=(tap == 0), stop=(tap == 8))
                        # interior-only relu copy: halos are never written so
                        # they stay zero from the initial memset
                        src_ap = ps[:].rearrange("p (r w) -> p r w", w=W)
                        d0 = GUARD + y0 * PW + 1
                        dst_ap = dst[:, d0:d0 + RW].rearrange(
                            "p (r w) -> p r w", w=PW)[:, :, 0:W]
                        kw = {}
                        if t == T - 1:
                            kw["accum_out"] = acc[:, r:r + 1]
                        if t < 0:
                            nc.scalar.activation(dst_ap, src_ap, Relu,
                                                 bias=stemb_sb[:], **kw)
                        else:
                            nc.scalar.activation(dst_ap, src_ap, Relu, **kw)

                pooled = sp.tile([128, 1], f32, tag="pooled")
                nc.vector.tensor_reduce(pooled[:], acc[:],
                                        axis=mybir.AxisListType.X,
                                        op=mybir.AluOpType.add)
                psfc = pp.tile([128, NT], f32, tag="ps", name="psfc")
                nc.tensor.matmul(psfc[0:IMGS * OC, 0:1], fcw_sb[:], pooled[:],
                                 start=True, stop=True)
                out_sb = sp.tile([IMGS * OC, 1], f32, tag="osb")
                nc.vector.tensor_add(out_sb[:], psfc[0:IMGS * OC, 0:1],
                                     fcb_sb[:])
                nc.sync.dma_start(d_out[g * IMGS:(g + 1) * IMGS, :], out_sb[:])

    nc.compile()
    _cache[repeat] = nc
    return nc


def _prep_shared(stem_weight, stem_bias, weight_schedule, fc_weight, fc_bias):
    """Host-side packing of the (replicated) weight tensors into SBUF layouts."""
    # per-step per-tap block-diagonal lhsT: [128,(img,ci) , 128,(img,co)]
    w = weight_schedule.astype(np.float32)          # [T, co, ci, 3, 3]
    lhs = np.zeros((T, 9, 128, 128), np.float16)
    for tap in range(9):
        dy, dx = tap // 3, tap % 3
        blk = np.transpose(w[:, :, :, dy, dx], (0, 2, 1))  # [T, ci, co]
        for i in range(IMGS):
            lhs[:, tap, i * HC:(i + 1) * HC, i * HC:(i + 1) * HC] = blk
    w_steps = np.ascontiguousarray(
        np.transpose(lhs, (2, 0, 1, 3)).reshape(128, T * 9 * 128))

    sw = stem_weight.astype(np.float32)             # [HC, 1, 3, 3]
    stem_lhs = np.zeros((IMGS * 9, 128), np.float16)
    for tap in range(9):
        dy, dx = tap // 3, tap % 3
        for i in range(IMGS):
            stem_lhs[i * 9 + tap, i * HC:(i + 1) * HC] = sw[:, 0, dy, dx]

    stem_b = np.tile(stem_bias.astype(np.float32), IMGS)[:, None].copy()

    fcw = np.zeros((128, IMGS * OC), np.float32)
    for i in range(IMGS):
        fcw[i * HC:(i + 1) * HC, i * OC:(i + 1) * OC] = \
            fc_weight.astype(np.float32).T / float(H * W)
    fc_b = np.tile(fc_bias.astype(np.float32), IMGS)[:, None].copy()

    return {"w_steps": w_steps, "stem_w": stem_lhs, "stem_b": stem_b,
            "fc_w": fcw, "fc_b": fc_b}


def _prep_xexp(x_imgs):
    """im2col-expand 8 images for the stem: [(img,tap), NINT] fp16.

    x_exp[(i,tap), j] = xpad[i, j + PW + off_tap] (flat padded coords,
    out-of-range reads are zero)."""
    xpad = np.zeros((IMGS, PW, PW), np.float32)
    xpad[:, 1:1 + H, 1:1 + W] = x_imgs[:, 0]
    flat = np.pad(xpad.reshape(IMGS, S), ((0, 0), (132, 132)))
    full = np.zeros((IMGS * 9, XEW), np.float16)
    for i in range(IMGS):
        for tap in range(9):
            base = 132 + PW + _TAP_OFF[tap]
            full[i * 9 + tap, :NINT] = flat[i, base:base + NINT]
    # chunked layout with 8 overlap columns so per-chunk views stay in-bounds
    out = np.zeros((IMGS * 9, XCH * (XCW + 8)), np.float16)
    for c in range(XCH):
        out[:, c * (XCW + 8):(c + 1) * (XCW + 8)] = \
            full[:, c * XCW:c * XCW + XCW + 8]
    return out


def kernel(x, stem_weight, stem_bias, weight_schedule, fc_weight, fc_bias):
    from concourse.bass_utils import run_bass_kernel_spmd

    x = np.asarray(x, dtype=np.float32)
    stem_weight = np.asarray(stem_weight, dtype=np.float32)
    stem_bias = np.asarray(stem_bias, dtype=np.float32)
    weight_schedule = np.asarray(weight_schedule, dtype=np.float32)
    fc_weight = np.asarray(fc_weight, dtype=np.float32)
    fc_bias = np.asarray(fc_bias, dtype=np.float32)

    nc = _build()
    shared = _prep_shared(stem_weight, stem_bias, weight_schedule,
                          fc_weight, fc_bias)
    in_maps = []
    for c in range(NCORES):
        m = dict(shared)
        for g in range(GROUPS):
            lo = c * GROUPS * IMGS + g * IMGS
            m[f"x_exp{g}"] = _prep_xexp(np.asarray(x[lo:lo + IMGS],
                                                   dtype=np.float32))
        in_maps.append(m)

    res = run_bass_kernel_spmd(nc, in_maps, core_ids=list(range(NCORES)),
                               trace=False)
    out = np.concatenate([res.results[c]["out"] for c in range(NCORES)], axis=0)
    return out.astype(np.float32)


# revision 20
# speedup vs baseline: 4.8794x; 1.0903x over previous
"""Trainium2 Bass kernel for CompiledNCA (stem conv -> 16 NCA conv steps -> pool -> fc).

Strategy: pure data parallel over batch (128 images -> 8 cores x 16 images).
Per core, images are processed in 2 groups of 8. SBUF layout puts
(image, channel) on the 128 partitions and the zero-padded 130x130 image
(flattened) on the free dim, so every 3x3 tap is a free-dim offset read.
Each conv step is 9 PSUM-accumulated matmuls with block-diagonal (8 x [16x16])
fp16 weights, using the full 128x128 PE array for 8 images at once.
ReLU + fp32->fp16 cast on ScalarE writes only interior pixels, so the zero
halo installed by an initial memset survives all steps. The final step's
activations also emit accum_out partial sums (free spatial pooling); the fc
layer is one tiny block-diagonal fp32 matmul + bias add.
"""

import numpy as np

B, HC, OC, T = 128, 16, 10, 16
H = W = 128
PW = H + 2                     # padded width/height: 130
S = PW * PW                    # 16900 flat padded pixels
NINT = H * PW                  # 16640 columns covering interior rows 1..128
XEW = NINT + 8                 # x_exp width (pad so 520-wide views stay in-bounds)
GUARD = 4                      # slack so shifted matmul reads stay in-bounds
HBUF = GUARD + S + 12          # h buffer free size (reads reach GUARD+S)
NCORES = 8
IMGS = 8                       # images per group (block-diag batch)
GROUPS = 2                     # groups per core
ROWS_PER_TILE = 4              # 4 rows x 128 interior cols -> N=512 (one bank)
Y0S = list(range(1, H + 1, ROWS_PER_TILE))   # 32 row-tiles over rows 1..128
NTILES = len(Y0S)
NT = ROWS_PER_TILE * W         # 512 matmul free size (interior only)
XCH = 4                        # x_exp DMA chunks per group (per-chunk deps)
XCW = NINT // XCH              # 4160 cols per chunk (+8 overlap columns)

_TAP_OFF = [(dy - 1) * PW + (dx - 1) for dy in range(3) for dx in range(3)]

_cache = {}


def _build(repeat=1):
    """Build + compile the Bass graph once; cached across kernel() calls.

    repeat > 1 duplicates the whole compute (same inputs/outputs) and is
    used only by the timing harness to measure marginal per-iteration
    device time within a single NEFF launch."""
    if repeat in _cache:
        return _cache[repeat]

    import concourse.bacc as bacc
    import concourse.mybir as mybir
    import concourse.tile as tile

    f16, f32 = mybir.dt.float16, mybir.dt.float32
    Relu = mybir.ActivationFunctionType.Relu

    nc = bacc.Bacc("TRN2", target_bir_lowering=False, debug=False,
                   enable_asserts=False, num_devices=NCORES)

    d_w = nc.dram_tensor("w_steps", [128, T * 9 * 128], f16, kind="ExternalInput")
    d_stemw = nc.dram_tensor("stem_w", [IMGS * 9, 128], f16, kind="ExternalInput")
    d_stemb = nc.dram_tensor("stem_b", [128, 1], f32, kind="ExternalInput")
    d_fcw = nc.dram_tensor("fc_w", [128, IMGS * OC], f32, kind="ExternalInput")
    d_fcb = nc.dram_tensor("fc_b", [IMGS * OC, 1], f32, kind="ExternalInput")
    d_xe = [nc.dram_tensor(f"x_exp{g}", [IMGS * 9, XCH * (XCW + 8)], f16,
                           kind="ExternalInput")
            for g in range(GROUPS)]
    d_out = nc.dram_tensor("out", [GROUPS * IMGS, OC], f32, kind="ExternalOutput")

    with tile.TileContext(nc) as tc:
        with tc.tile_pool(name="const", bufs=1) as cp, \
             tc.tile_pool(name="hbuf", bufs=1) as hp, \
             tc.tile_pool(name="small", bufs=2) as sp, \
             tc.tile_pool(name="psum", bufs=8, space="PSUM") as pp:

            # DMA order matters for the startup stall: the stem needs only
            # stem_w/stem_b + the first x_exp chunk, so those go first and
            # get their own tiles (per-tile deps); the 4.7MB step-weight DMA
            # is split per step and queued behind.
            stemw_sb = cp.tile([IMGS * 9, 128], f16, tag="sw")
            nc.sync.dma_start(stemw_sb[:], d_stemw[:])
            stemb_sb = cp.tile([128, 1], f32, tag="sb")
            nc.sync.dma_start(stemb_sb[:], d_stemb[:])

            def load_xe_chunk(g, c):
                t_ = cp.tile([IMGS * 9, XCW + 8], f16, tag=f"xe{g}_{c}",
                             name=f"xe{g}_{c}")
                nc.sync.dma_start(
                    t_[:], d_xe[g][:, c * (XCW + 8):(c + 1) * (XCW + 8)])
                return t_

            def load_w(t):
                t_ = cp.tile([128, 9 * 128], f16, tag=f"w{t}", name=f"w{t}")
                nc.sync.dma_start(t_[:], d_w[:, t * 9 * 128:(t + 1) * 9 * 128])
                return t_

            # issue order tracks first-use time: stem consumes xe0 chunks
            # from ~2us, step 0 needs w0 at ~8us, step t at ~8+t*61us,
            # group 1's xe at ~1ms.
            xe_sb = [[load_xe_chunk(0, 0), load_xe_chunk(0, 1)], []]
            w_tiles = [load_w(0)]
            for c in range(2, XCH):
                xe_sb[0].append(load_xe_chunk(0, c))
            for t in range(1, T):
                w_tiles.append(load_w(t))
            for c in range(XCH):
                xe_sb[1].append(load_xe_chunk(1, c))
            fcw_sb = cp.tile([128, IMGS * OC], f32, tag="fw")
            nc.sync.dma_start(fcw_sb[:], d_fcw[:])
            fcb_sb = cp.tile([IMGS * OC, 1], f32, tag="fb")
            nc.sync.dma_start(fcb_sb[:], d_fcb[:])

            hbufs = [hp.tile([128, HBUF], f16, tag=f"h{i}", name=f"h{i}")
                     for i in range(2)]
            # zero only the cells the relu copies never write: guards, the
            # top/bottom halo rows, and the 2-col halo seams between rows.
            # The interior is overwritten every step, so a full-buffer memset
            # (~17us on DVE) would only stall the stem.
            for hb in hbufs:
                nc.vector.memset(hb[:, 0:GUARD + PW + 1], 0.0)
                s0 = GUARD + PW + W + 1          # right halo of row 1
                seams = hb[:, s0:s0 + (H - 1) * PW].rearrange(
                    "p (y two) -> p y two", two=PW)[:, :, 0:2]
                nc.vector.memset(seams, 0.0)
                nc.vector.memset(hb[:, GUARD + H * PW + PW - 1:HBUF], 0.0)

            for _rep in range(repeat):
              for g in range(GROUPS):
                acc = sp.tile([128, NTILES], f32, tag="acc")
                # t = -1 is the stem (im2col-expanded x, one matmul pass);
                # t in [0, T) are the NCA steps (9 taps each).
                for t in range(-1, T):
                    src = hbufs[t % 2]
                    dst = hbufs[(t + 1) % 2]
                    for r, y0 in enumerate(Y0S):
                        RW = ROWS_PER_TILE * PW  # 520-col span for 3D views
                        ps = pp.tile([128, NT], f32, tag="ps")
                        if t < 0:
                            c = r // (NTILES // XCH)
                            j0 = (y0 - 1) * PW + 1 - c * XCW
                            rhs = xe_sb[g][c][:, j0:j0 + RW].rearrange(
                                "p (r w) -> p r w", w=PW)[:, :, 0:W]
                            nc.tensor.matmul(ps[:], stemw_sb[:], rhs,
                                             start=True, stop=True)
                        else:
                            for tap in range(9):
                                a = GUARD + y0 * PW + 1 + _TAP_OFF[tap]
                                rhs = src[:, a:a + RW].rearrange(
                                    "p (r w) -> p r w", w=PW)[:, :, 0:W]
                                wv = w_tiles[t][:, tap * 128:(tap + 1) * 128]
                                nc.tensor.matmul(
                                    ps[:], wv, rhs,
                                    start=(tap == 0), stop=(tap == 8))
                        # interior-only relu copy: halos are never written so
                        # they stay zero from the initial memset
                        src_ap = ps[:].rearrange("p (r w) -> p r w", w=W)
                        d0 = GUARD + y0 * PW + 1
                        dst_ap = dst[:, d0:d0 + RW].rearrange(
                            "p (r w) -> p r w", w=PW)[:, :, 0:W]
                        if t < 0:
                            nc.scalar.activation(dst_ap, src_ap, Relu,
                                                 bias=stemb_sb[:])
                        elif t == T - 1:
                            nc.scalar.activation(dst_ap, src_ap, Relu,
                                                 accum_out=acc[:, r:r + 1])
                        elif r % 2:
                            # split relu copies across engines: odd tiles on
                            # DVE, even on ACT
                            nc.vector.tensor_scalar_max(dst_ap, src_ap, 0.0)
                        else:
                            nc.scalar.activation(dst_ap, src_ap, Relu)

                pooled = sp.tile([128, 1], f32, tag="pooled")
                nc.vector.tensor_reduce(pooled[:], acc[:],
                                        axis=mybir.AxisListType.X,
                                        op=mybir.AluOpType.add)
                psfc = pp.tile([128, NT], f32, tag="ps", name="psfc")
                nc.tensor.matmul(psfc[0:IMGS * OC, 0:1], fcw_sb[:], pooled[:],
                                 start=True, stop=True)
                out_sb = sp.tile([IMGS * OC, 1], f32, tag="osb")
                nc.vector.tensor_add(out_sb[:], psfc[0:IMGS * OC, 0:1],
                                     fcb_sb[:])
                nc.sync.dma_start(d_out[g * IMGS:(g + 1) * IMGS, :], out_sb[:])

    nc.compile()
    _cache[repeat] = nc
    return nc


def _prep_shared(stem_weight, stem_bias, weight_schedule, fc_weight, fc_bias):
    """Host-side packing of the (replicated) weight tensors into SBUF layouts."""
    # per-step per-tap block-diagonal lhsT: [128,(img,ci) , 128,(img,co)]
    w = weight_schedule.astype(np.float32)          # [T, co, ci, 3, 3]
    lhs = np.zeros((T, 9, 128, 128), np.float16)
    for tap in range(9):
        dy, dx = tap // 3, tap % 3
        blk = np.transpose(w[:, :, :, dy, dx], (0, 2, 1))  # [T, ci, co]
        for i in range(IMGS):
            lhs[:, tap, i * HC:(i + 1) * HC, i * HC:(i + 1) * HC] = blk
    w_steps = np.ascontiguousarray(
        np.transpose(lhs, (2, 0, 1, 3)).reshape(128, T * 9 * 128))

    sw = stem_weight.astype(np.float32)             # [HC, 1, 3, 3]
    stem_lhs = np.zeros((IMGS * 9, 128), np.float16)
    for tap in range(9):
        dy, dx = tap // 3, tap % 3
        for i in range(IMGS):
            stem_lhs[i * 9 + tap, i * HC:(i + 1) * HC] = sw[:, 0, dy, dx]

    stem_b = np.tile(stem_bias.astype(np.float32), IMGS)[:, None].copy()

    fcw = np.zeros((128, IMGS * OC), np.float32)
    for i in range(IMGS):
        fcw[i * HC:(i + 1) * HC, i * OC:(i + 1) * OC] = \
            fc_weight.astype(np.float32).T / float(H * W)
    fc_b = np.tile(fc_bias.astype(np.float32), IMGS)[:, None].copy()

    return {"w_steps": w_steps, "stem_w": stem_lhs, "stem_b": stem_b,
            "fc_w": fcw, "fc_b": fc_b}


def _prep_xexp(x_imgs):
    """im2col-expand 8 images for the stem: [(img,tap), NINT] fp16.

    x_exp[(i,tap), j] = xpad[i, j + PW + off_tap] (flat padded coords,
    out-of-range reads are zero)."""
    xpad = np.zeros((IMGS, PW, PW), np.float32)
    xpad[:, 1:1 + H, 1:1 + W] = x_imgs[:, 0]
    flat = np.pad(xpad.reshape(IMGS, S), ((0, 0), (132, 132)))
    full = np.zeros((IMGS * 9, XEW), np.float16)
    for i in range(IMGS):
        for tap in range(9):
            base = 132 + PW + _TAP_OFF[tap]
            full[i * 9 + tap, :NINT] = flat[i, base:base + NINT]
    # chunked layout with 8 overlap columns so per-chunk views stay in-bounds
    out = np.zeros((IMGS * 9, XCH * (XCW + 8)), np.float16)
    for c in range(XCH):
        out[:, c * (XCW + 8):(c + 1) * (XCW + 8)] = \
            full[:, c * XCW:c * XCW + XCW + 8]
    return out


def kernel(x, stem_weight, stem_bias, weight_schedule, fc_weight, fc_bias):
    from concourse.bass_utils import run_bass_kernel_spmd

    x = np.asarray(x, dtype=np.float32)
    stem_weight = np.asarray(stem_weight, dtype=np.float32)
    stem_bias = np.asarray(stem_bias, dtype=np.float32)
    weight_schedule = np.asarray(weight_schedule, dtype=np.float32)
    fc_weight = np.asarray(fc_weight, dtype=np.float32)
    fc_bias = np.asarray(fc_bias, dtype=np.float32)

    nc = _build()
    shared = _prep_shared(stem_weight, stem_bias, weight_schedule,
                          fc_weight, fc_bias)
    in_maps = []
    for c in range(NCORES):
        m = dict(shared)
        for g in range(GROUPS):
            lo = c * GROUPS * IMGS + g * IMGS
            m[f"x_exp{g}"] = _prep_xexp(np.asarray(x[lo:lo + IMGS],
                                                   dtype=np.float32))
        in_maps.append(m)

    res = run_bass_kernel_spmd(nc, in_maps, core_ids=list(range(NCORES)),
                               trace=False)
    out = np.concatenate([res.results[c]["out"] for c in range(NCORES)], axis=0)
    return out.astype(np.float32)
